# revision 9
# baseline (speedup 1.0000x reference)
# Trainium2 Bass kernel for nn_CoefficientLayer (per-species MLP dispatch,
# ANI-style).
#
# v5: species s -> cores 2s, 2s+1 (routing and weight-folding as v4).  The
# device schedule is rebuilt around three ideas:
#
# 1. PE slot packing (13 N=512 slots/tile, was 16, grouped by array mode to
#    minimize reconfig drains):
#      ROW slot : 4 K=1 bias ("aug") matmuls at row groups 0/32/64/96
#                 (L1 m0/m1, L2 m0/m1), each into a different PSUM bank,
#                 emitted FIRST with start=True so they clear their banks.
#      FULL x10 : L2m0 k0/k1; L3m0 k0; L3m0 k1 zero-padded to K=128 so it
#                 streams the shared `hm1` tile (rows 0:32 are constant 1.0
#                 -> L3's bias rides weight row 0 for free); L1 x6.
#      COL  x2  : column-tiled trio sharing one PSUM bank at disjoint
#                 partitions: L4 -> [0:1] (col 0), L3m1 -> [32:64] (col 32),
#                 L2m1 -> [64:128] (col 64).  k1 members stream `hm1`
#                 (1.0-rows carry L3m1's bias and L4's folded shifter bias).
# 2. psm1: one persistent [P,2,512] f32 tile (2 banks, even/odd tiles) holds
#    L2m1(t+2) + L3m1(t+1) + L4(t); a single Exp/stt pair post-processes
#    [32:128]; the L4 row [0:1] is evacuated to SBUF by a copy that
#    alternates ScalarE/VectorE per tile to balance engine load.
# 3. Merged posts: L2m0(t+2) and L3m0(t+1) share one [P,2,512] psum tile so
#    one Exp + one stt covers both (saves inst overhead on both engines).
#
# Math (e = exp(1)): stored Hb_k = e*(elu(y_k)+1), y_k = x_k/alpha:
#   psum = e*(y+1) (weight chunks + aug rank-1 updates)
#   ACT:  E = Exp(psum/e);   DVE:  Hb = (E min e) max psum
# L4: psum4 = (s1*alpha/e)*W4^T Hb3 + ride(s0 + s1*beta4) = final output.
import numpy as np
from contextlib import ExitStack

import concourse.bass as bass
import concourse.tile as tile
from concourse import bacc, mybir
from concourse.bass_utils import run_bass_kernel_spmd

ALPHA = 0.1
E1 = float(np.exp(1.0))
P = 128
NCORES = 8
QUANTUM = 128
DIN = 384

F32 = mybir.dt.float32
BF16 = mybir.dt.bfloat16
AF = mybir.ActivationFunctionType
ALU = mybir.AluOpType

# weight image column layout (per species, [128, WCOLS] bf16)
AUG = 0            # 128 cols: row0=aug1[0:128] row32=aug1[128:256]
                   #           row64=aug2[0:128] row96=aug2[128:192] (64 cols)
L1C = 128          # 6 x 128: (m,k) -> L1C + (m*3+k)*128
L2C = L1C + 6 * P  # 2 x 128: L2m0 k
L3K0 = L2C + 2 * P   # 128: W3p[0:128, 0:128]
FK1 = L3K0 + P       # 128: row1=aug3[0:128], rows64:128=W3p[128:192, 0:128]
EK0 = FK1 + P        # 64: W2p[0:128, 128:192]
EK1 = EK0 + 64       # 64: W2p[128:256, 128:192]
GK0 = EK1 + 64       # 32: W3p[0:128, 128:160]
GK1 = GK0 + 32       # 32: row1=aug3[128:160], rows64:128=W3p[128:192,128:160]
IK0 = GK1 + 32       # 32: col0=W4p[0:128], cols1:32 zero (zeros psm1[1:32])
IK1 = IK0 + 32       # 32: col0: row1=s0+s1*beta4, rows32:64=W4p[128:160]
Z32 = IK1 + 32       # 32 zero cols (warmup dummy clear of psm1[0:32])
WCOLS = Z32 + 32
WSPLIT = L2C       # prologue DMA-A covers [0:WSPLIT) = AUG + L1


def _fold_host(inputs):
    """Per-species weight image [4][128, WCOLS] f32."""
    al = ALPHA
    wimgs = []
    for s in range(4):
        W = [np.asarray(inputs[f"W{i}"][s], np.float32) for i in (1, 2, 3, 4)]
        b = [np.asarray(inputs[f"b{i}"][s], np.float32) for i in (1, 2, 3, 4)]
        s1 = float(np.asarray(inputs["shift_b1"], np.float32)[s])
        s0 = float(np.asarray(inputs["shift_b0"], np.float32)[s])
        W1p = (E1 / al) * W[0]
        W2p, W3p = W[1], W[2]
        W4p = (s1 * al / E1) * W[3]
        aug1 = E1 * (b[0] / al + 1.0)
        aug2 = E1 * (b[1] / al - W[1].sum(0) + 1.0)
        aug3 = E1 * (b[2] / al - W[2].sum(0) + 1.0)
        bias4 = s0 + s1 * float((b[3] - al * W[3].sum(0))[0])

        w = np.zeros((P, WCOLS), dtype=np.float32)
        w[0, AUG:AUG + 128] = aug1[:128]
        w[32, AUG:AUG + 128] = aug1[128:]
        w[64, AUG:AUG + 128] = aug2[:128]
        w[96, AUG:AUG + 64] = aug2[128:]
        for m in range(2):
            for k in range(3):
                c = L1C + (m * 3 + k) * P
                w[:, c:c + P] = W1p[k * P:(k + 1) * P, m * P:(m + 1) * P]
        for k in range(2):
            w[:, L2C + k * P:L2C + (k + 1) * P] = W2p[k * P:(k + 1) * P, :P]
        w[:, L3K0:L3K0 + P] = W3p[:P, :P]
        w[1, FK1:FK1 + P] = aug3[:128]
        w[64:, FK1:FK1 + P] = W3p[P:, :P]
        w[:, EK0:EK0 + 64] = W2p[:P, P:]
        w[:, EK1:EK1 + 64] = W2p[P:, P:]
        w[:, GK0:GK0 + 32] = W3p[:P, P:]
        w[1, GK1:GK1 + 32] = aug3[128:]
        w[64:, GK1:GK1 + 32] = W3p[P:, P:]
        w[:, IK0] = W4p[:P, 0]
        w[1, IK1] = bias4
        w[32:64, IK1] = W4p[P:, 0]
        wimgs.append(w)
    return wimgs


def _make_sched(A_pc):
    """Non-increasing tile sizes: 512s then one runt."""
    tiles, col, rem = [], 0, A_pc
    while rem >= 512:
        tiles.append((col, 512))
        col += 512
        rem -= 512
    if rem:
        tiles.append((col, rem))
    return tiles


def _host_prepare(inputs):
    species = np.asarray(inputs["species"]).ravel()
    aev = np.ascontiguousarray(
        np.asarray(inputs["aev"], np.float32).reshape(-1, DIN))
    order = np.argsort(species, kind="stable")
    counts = np.bincount(species, minlength=4)
    loads = []
    for s in range(4):
        h = (int(counts[s]) + 1) // 2
        loads += [h, int(counts[s]) - h]
    A_pc = max(512, int(np.ceil(max(loads) / QUANTUM)) * QUANTUM)

    idx = np.full((NCORES, A_pc), -1, dtype=np.int64)
    off = 0
    for s in range(4):
        grp = order[off:off + counts[s]]
        off += counts[s]
        h = (int(counts[s]) + 1) // 2
        idx[2 * s, :h] = grp[:h]
        idx[2 * s + 1, :counts[s] - h] = grp[h:]

    import ml_dtypes
    aev_t = np.zeros((NCORES, DIN, A_pc), dtype=ml_dtypes.bfloat16)
    for c in range(NCORES):
        valid = idx[c] >= 0
        aev_t[c][:, valid] = aev[idx[c][valid]].T.astype(ml_dtypes.bfloat16)

    sched = _make_sched(A_pc)
    return aev_t, idx, sched, A_pc


def _build_program(sched, A_pc):
    nc = bacc.Bacc("TRN2", target_bir_lowering=False, debug=False)
    aev_d = nc.dram_tensor("aev_t", [DIN, A_pc], BF16, kind="ExternalInput").ap()
    w_d = nc.dram_tensor("wimg", [P, WCOLS], BF16, kind="ExternalInput").ap()
    out_d = nc.dram_tensor("out", [1, A_pc], F32, kind="ExternalOutput").ap()
    T = len(sched)

    with tile.TileContext(nc) as tc, ExitStack() as ctx:
        spool = ctx.enter_context(tc.tile_pool(name="s", bufs=1))
        xpool = ctx.enter_context(tc.tile_pool(name="x", bufs=5))
        hpool = ctx.enter_context(tc.tile_pool(name="h", bufs=3))
        epool = ctx.enter_context(tc.tile_pool(name="e", bufs=2))
        pspool = ctx.enter_context(tc.tile_pool(name="ps", bufs=1, space="PSUM"))

        wt = spool.tile([P, WCOLS], BF16, tag="wt")
        nc.sync.dma_start(wt[:, :WSPLIT], w_d[:, :WSPLIT])

        ones = spool.tile([P, 512], BF16, tag="ones")
        nc.vector.memset(ones[:], 1.0)
        hm1 = [spool.tile([P, 512], BF16, tag=f"hm1{i}", name=f"hm1{i}")
               for i in range(2)]
        nc.vector.memset(hm1[0][:], 1.0)
        nc.vector.memset(hm1[1][:], 1.0)
        ystage = spool.tile([1, A_pc], F32, tag="ystage")
        # touch Exp so the ACT table load is off the first tile's critical path
        scratch = spool.tile([1, 1], F32, tag="scratch")
        nc.scalar.activation(scratch[:], ones[0:1, 0:1], AF.Exp,
                             bias=0.0, scale=1.0)

        xloads = {}

        def stage_load(u):
            col, n = sched[u]
            xt = xpool.tile([P, 3, 512], BF16, tag="x")
            src = aev_d.rearrange("(k p) a -> p k a", k=3)
            nc.sync.dma_start(xt[:, :, :n], src[:, :, col:col + n])
            xloads[u] = xt

        stage_load(0)
        if T > 1:
            stage_load(1)
        nc.sync.dma_start(wt[:, WSPLIT:], w_d[:, WSPLIT:])

        psm1 = pspool.tile([P, 2, 512], F32, tag="psm1")
        hid = {}      # (tile, layer) -> h tile
        h23_of = {}   # iteration -> h23 tile
        MM = nc.tensor.matmul

        def emit_augs(i, ps1, ps23):
            """One ROW-mode slot: up to 4 concurrent K=1 aug matmuls at rows
            0/32/64/96, each into a different PSUM bank, start=True (clears
            their banks; mains follow with start=False)."""
            t4, t2 = i + 4, i + 2
            if ps1 is not None:
                _, n4 = sched[t4]
                MM(ps1[:, 0, :n4], wt[0:1, AUG:AUG + P], ones[0:1, :n4],
                   start=True, stop=False, tile_position=(0, 0))
                MM(ps1[:, 1, :n4], wt[32:33, AUG:AUG + P], ones[32:33, :n4],
                   start=True, stop=False, tile_position=(32, 0))
            if 0 <= t2 < T:
                _, n2 = sched[t2]
                MM(ps23[:, 0, :n2], wt[64:65, AUG:AUG + P], ones[64:65, :n2],
                   start=True, stop=False, tile_position=(64, 0))
                MM(psm1[64:128, i % 2, :n2], wt[96:97, AUG:AUG + 64],
                   ones[96:97, :n2], start=True, stop=False,
                   tile_position=(96, 64))

        def emit_l1_mains(i, ps1):
            t4 = i + 4
            _, n4 = sched[t4]
            xt = xloads.pop(t4)
            for m in range(2):
                for k in range(3):
                    MM(ps1[:, m, :n4], wt[:, L1C + (m * 3 + k) * P:
                                          L1C + (m * 3 + k + 1) * P],
                       xt[:, k, :n4], start=False, stop=(k == 2))

        def emit_full_23(i, ps23):
            """FULL-mode L2m0(t2) + L3m0(t1) including hm1-ride k1."""
            t2, t1 = i + 2, i + 1
            if 0 <= t2 < T:
                _, n2 = sched[t2]
                h1 = hid[(t2, 1)]
                for k in range(2):
                    MM(ps23[:, 0, :n2], wt[:, L2C + k * P:L2C + (k + 1) * P],
                       h1[:, k, :n2], start=False, stop=(k == 1))
            if 0 <= t1 < T:
                _, n1 = sched[t1]
                h23p = h23_of[i - 1]
                MM(ps23[:, 1, :n1], wt[:, L3K0:L3K0 + P], h23p[:, 0, :n1],
                   start=True, stop=False)
                MM(ps23[:, 1, :n1], wt[:, FK1:FK1 + P],
                   hm1[(i - 1) % 2][:, :n1], start=False, stop=True)

        def emit_col(i):
            """COL-mode trio: L2m1(t2)@64:128, L3m1(t1)@32:64, L4(t0)@0:1."""
            t2, t1, t0 = i + 2, i + 1, i
            b = i % 2
            if 0 <= t2 < T:
                _, n2 = sched[t2]
                h1 = hid[(t2, 1)]
                MM(psm1[64:128, b, :n2], wt[:, EK0:EK0 + 64], h1[:, 0, :n2],
                   start=False, stop=False, tile_position=(0, 64))
            if 0 <= t1 < T:
                _, n1 = sched[t1]
                h23p = h23_of[i - 1]
                MM(psm1[32:64, b, :n1], wt[:, GK0:GK0 + 32], h23p[:, 0, :n1],
                   start=True, stop=False, tile_position=(0, 32))
            if 0 <= t0 < T:
                _, n0 = sched[t0]
                h23p = h23_of[i - 1]
                MM(psm1[0:32, b, :n0], wt[:, IK0:IK0 + 32], h23p[:, 1, :n0],
                   start=True, stop=False, tile_position=(0, 0))
            else:
                # warmup: zero psm1[0:32] so the full-span stt makes exact
                # 1.0 pad rows (rides live on hm1 row 1)
                nz = max(sched[t][1] for t in (t2, t1) if 0 <= t < T)
                MM(psm1[0:32, b, :nz], wt[:, Z32:Z32 + 32], ones[:, :nz],
                   start=True, stop=True, tile_position=(0, 0))
            # k1 members (stream hm1: 1.0-rows carry aug3m1 / shifter bias)
            if 0 <= t2 < T:
                _, n2 = sched[t2]
                h1 = hid.pop((t2, 1))
                MM(psm1[64:128, b, :n2], wt[:, EK1:EK1 + 64], h1[:, 1, :n2],
                   start=False, stop=True, tile_position=(0, 64))
            if 0 <= t1 < T:
                _, n1 = sched[t1]
                MM(psm1[32:64, b, :n1], wt[:, GK1:GK1 + 32],
                   hm1[(i - 1) % 2][:, :n1], start=False, stop=True,
                   tile_position=(0, 32))
            if 0 <= t0 < T:
                _, n0 = sched[t0]
                MM(psm1[0:32, b, :n0], wt[:, IK1:IK1 + 32],
                   hm1[(i - 1) % 2][:, :n0], start=False, stop=True,
                   tile_position=(0, 0))

        def post_m1(i):
            """Exp + stt over the full psm1 bank -> hm1[i%2].  Rows 1:32
            are zero in psum, so stt yields exact 1.0 pad rows there (the
            bias rides); row 0 (L4 out) becomes garbage in hm1 but is only
            ever streamed against zero weight rows."""
            b = i % 2
            em1 = epool.tile([P, 512], BF16, tag="em1")
            nc.scalar.activation(em1[:, :], psm1[:, b, :], AF.Exp,
                                 bias=0.0, scale=1.0 / E1)
            nc.vector.scalar_tensor_tensor(
                hm1[b][:, :], em1[:, :], E1, psm1[:, b, :],
                ALU.min, ALU.max)

        def post_23(i, ps23):
            t2, t1 = i + 2, i + 1
            b0 = 0 if 0 <= t2 < T else 1
            b1 = 2 if 0 <= t1 < T else 1
            if b0 >= b1:
                return
            n = max(sched[t][1] for t in (t2, t1) if 0 <= t < T)
            e23 = epool.tile([P, 2, 512], BF16, tag="e23")
            nc.scalar.activation(e23[:, b0:b1, :n], ps23[:, b0:b1, :n], AF.Exp,
                                 bias=0.0, scale=1.0 / E1)
            h23 = hpool.tile([P, 2, 512], BF16, tag="h23")
            nc.vector.scalar_tensor_tensor(
                h23[:, b0:b1, :n], e23[:, b0:b1, :n], E1, ps23[:, b0:b1, :n],
                ALU.min, ALU.max)
            h23_of[i] = h23

        def post_l1(i, ps1):
            t4 = i + 4
            _, n4 = sched[t4]
            e1 = epool.tile([P, 2, 512], BF16, tag="e1")
            nc.scalar.activation(e1[:, :, :n4], ps1[:, :, :n4], AF.Exp,
                                 bias=0.0, scale=1.0 / E1)
            h1 = hpool.tile([P, 2, 512], BF16, tag="h1")
            nc.vector.scalar_tensor_tensor(
                h1[:, :, :n4], e1[:, :, :n4], E1, ps1[:, :, :n4],
                ALU.min, ALU.max)
            hid[(t4, 1)] = h1

        def evac(i):
            col, n = sched[i]
            src = psm1[0:1, i % 2, :n]
            dst = ystage[0:1, col:col + n]
            if i % 2 == 0:
                nc.vector.tensor_scalar_add(dst, src, 0.0)
            else:
                nc.scalar.activation(dst, src, AF.Identity, bias=0.0,
                                     scale=1.0)
            nc.sync.dma_start(out_d[:, col:col + n], ystage[:, col:col + n])

        for i in range(-4, T):
            t4, t2, t1, t0 = i + 4, i + 2, i + 1, i
            if 2 <= i + 6 < T:
                stage_load(i + 6)
            ps1 = None
            if 0 <= t4 < T:
                ps1 = pspool.tile([P, 2, 512], F32, tag="ps1", bufs=2)
            ps23 = None
            if 0 <= t2 < T or 0 <= t1 < T:
                ps23 = pspool.tile([P, 2, 512], F32, tag="ps23")
            emit_augs(i, ps1, ps23)
            if i < 0:
                # warmup: L1 first so its ready matmuls aren't queued behind
                # an L2 that waits on h1
                if ps1 is not None:
                    emit_l1_mains(i, ps1)
                if ps23 is not None:
                    emit_full_23(i, ps23)
            else:
                if ps23 is not None:
                    emit_full_23(i, ps23)
                if ps1 is not None:
                    emit_l1_mains(i, ps1)
            if 0 <= t2 < T or 0 <= t1 < T or 0 <= t0 < T:
                emit_col(i)
            if ps23 is not None:
                post_23(i, ps23)
            if 0 <= t2 < T or 0 <= t1 < T:
                post_m1(i)
            if ps1 is not None:
                post_l1(i, ps1)
            if 0 <= t0 < T:
                evac(i)

    nc.compile()
    return nc


def kernel(**inputs):
    import ml_dtypes
    species = np.asarray(inputs["species"])
    out_dtype = np.asarray(inputs["aev"]).dtype
    aev_t, idx, sched, A_pc = _host_prepare(inputs)
    wimgs = _fold_host(inputs)
    nc = _build_program(sched, A_pc)

    in_maps = [{"aev_t": np.ascontiguousarray(aev_t[c]),
                "wimg": wimgs[c // 2].astype(ml_dtypes.bfloat16)}
               for c in range(NCORES)]
    res = run_bass_kernel_spmd(nc, in_maps, core_ids=list(range(NCORES)))

    out = np.zeros(species.size, dtype=np.float32)
    for c in range(NCORES):
        valid = idx[c] >= 0
        out[idx[c][valid]] = res.results[c]["out"][0][valid]
    return out.reshape(species.shape).astype(out_dtype, copy=False)


# revision 10
# speedup vs baseline: 1.0967x; 1.0967x over previous
# Trainium2 Bass kernel for nn_CoefficientLayer (per-species MLP dispatch,
# ANI-style).
#
# v5: species s -> cores 2s, 2s+1 (routing and weight-folding as v4).  The
# device schedule is rebuilt around three ideas:
#
# 1. PE slot packing (13 N=512 slots/tile, was 16, grouped by array mode to
#    minimize reconfig drains):
#      ROW slot : 4 K=1 bias ("aug") matmuls at row groups 0/32/64/96
#                 (L1 m0/m1, L2 m0/m1), each into a different PSUM bank,
#                 emitted FIRST with start=True so they clear their banks.
#      FULL x10 : L2m0 k0/k1; L3m0 k0; L3m0 k1 zero-padded to K=128 so it
#                 streams the shared `hm1` tile (rows 0:32 are constant 1.0
#                 -> L3's bias rides weight row 0 for free); L1 x6.
#      COL  x2  : column-tiled trio sharing one PSUM bank at disjoint
#                 partitions: L4 -> [0:1] (col 0), L3m1 -> [32:64] (col 32),
#                 L2m1 -> [64:128] (col 64).  k1 members stream `hm1`
#                 (1.0-rows carry L3m1's bias and L4's folded shifter bias).
# 2. psm1: one persistent [P,2,512] f32 tile (2 banks, even/odd tiles) holds
#    L2m1(t+2) + L3m1(t+1) + L4(t); a single Exp/stt pair post-processes
#    [32:128]; the L4 row [0:1] is evacuated to SBUF by a copy that
#    alternates ScalarE/VectorE per tile to balance engine load.
# 3. Merged posts: L2m0(t+2) and L3m0(t+1) share one [P,2,512] psum tile so
#    one Exp + one stt covers both (saves inst overhead on both engines).
#
# Math (e = exp(1)): stored Hb_k = e*(elu(y_k)+1), y_k = x_k/alpha:
#   psum = e*(y+1) (weight chunks + aug rank-1 updates)
#   ACT:  E = Exp(psum/e);   DVE:  Hb = (E min e) max psum
# L4: psum4 = (s1*alpha/e)*W4^T Hb3 + ride(s0 + s1*beta4) = final output.
import numpy as np
from contextlib import ExitStack

import concourse.bass as bass
import concourse.tile as tile
from concourse import bacc, mybir
from concourse.bass_utils import run_bass_kernel_spmd

ALPHA = 0.1
E1 = float(np.exp(1.0))
P = 128
NCORES = 8
QUANTUM = 128
DIN = 384

F32 = mybir.dt.float32
BF16 = mybir.dt.bfloat16
AF = mybir.ActivationFunctionType
ALU = mybir.AluOpType

# weight image column layout (per species, [128, WCOLS] bf16)
AUG = 0            # 128 cols: row0=aug1[0:128] row32=aug1[128:256]
                   #           row64=aug2[0:128] row96=aug2[128:192] (64 cols)
L1C = 128          # 6 x 128: (m,k) -> L1C + (m*3+k)*128
L2C = L1C + 6 * P  # 2 x 128: L2m0 k
L3K0 = L2C + 2 * P   # 128: W3p[0:128, 0:128]
FK1 = L3K0 + P       # 128: row1=aug3[0:128], rows64:128=W3p[128:192, 0:128]
EK0 = FK1 + P        # 64: W2p[0:128, 128:192]
EK1 = EK0 + 64       # 64: W2p[128:256, 128:192]
GK0 = EK1 + 64       # 32: W3p[0:128, 128:160]
GK1 = GK0 + 32       # 32: row1=aug3[128:160], rows64:128=W3p[128:192,128:160]
IK0 = GK1 + 32       # 32: col0=W4p[0:128], cols1:32 zero (zeros psm1[1:32])
IK1 = IK0 + 32       # 32: col0: row1=s0+s1*beta4, rows32:64=W4p[128:160]
A2M1 = IK1 + 32      # 64: row0=aug2[128:192] (two 32-col halves, col-mode)
Z32 = A2M1 + 64      # 32 zero cols (warmup dummy clear of psm1[0:32])
WCOLS = Z32 + 32
WSPLIT = L2C       # prologue DMA-A covers [0:WSPLIT) = AUG + L1


def _fold_host(inputs):
    """Per-species weight image [4][128, WCOLS] f32."""
    al = ALPHA
    wimgs = []
    for s in range(4):
        W = [np.asarray(inputs[f"W{i}"][s], np.float32) for i in (1, 2, 3, 4)]
        b = [np.asarray(inputs[f"b{i}"][s], np.float32) for i in (1, 2, 3, 4)]
        s1 = float(np.asarray(inputs["shift_b1"], np.float32)[s])
        s0 = float(np.asarray(inputs["shift_b0"], np.float32)[s])
        W1p = (E1 / al) * W[0]
        W2p, W3p = W[1], W[2]
        W4p = (s1 * al / E1) * W[3]
        aug1 = E1 * (b[0] / al + 1.0)
        aug2 = E1 * (b[1] / al - W[1].sum(0) + 1.0)
        aug3 = E1 * (b[2] / al - W[2].sum(0) + 1.0)
        bias4 = s0 + s1 * float((b[3] - al * W[3].sum(0))[0])

        w = np.zeros((P, WCOLS), dtype=np.float32)
        w[0, AUG:AUG + 128] = aug1[:128]
        w[32, AUG:AUG + 128] = aug1[128:]
        w[64, AUG:AUG + 128] = aug2[:128]
        w[0, A2M1:A2M1 + 64] = aug2[128:]
        for m in range(2):
            for k in range(3):
                c = L1C + (m * 3 + k) * P
                w[:, c:c + P] = W1p[k * P:(k + 1) * P, m * P:(m + 1) * P]
        for k in range(2):
            w[:, L2C + k * P:L2C + (k + 1) * P] = W2p[k * P:(k + 1) * P, :P]
        w[:, L3K0:L3K0 + P] = W3p[:P, :P]
        w[1, FK1:FK1 + P] = aug3[:128]
        w[64:, FK1:FK1 + P] = W3p[P:, :P]
        w[:, EK0:EK0 + 64] = W2p[:P, P:]
        w[:, EK1:EK1 + 64] = W2p[P:, P:]
        w[:, GK0:GK0 + 32] = W3p[:P, P:]
        w[1, GK1:GK1 + 32] = aug3[128:]
        w[64:, GK1:GK1 + 32] = W3p[P:, P:]
        w[:, IK0] = W4p[:P, 0]
        w[1, IK1] = bias4
        w[32:64, IK1] = W4p[P:, 0]
        wimgs.append(w)
    return wimgs


def _make_sched(A_pc):
    """Non-increasing tile sizes: 512s then one runt."""
    tiles, col, rem = [], 0, A_pc
    while rem >= 512:
        tiles.append((col, 512))
        col += 512
        rem -= 512
    if rem:
        tiles.append((col, rem))
    return tiles


def _host_prepare(inputs):
    species = np.asarray(inputs["species"]).ravel()
    aev = np.ascontiguousarray(
        np.asarray(inputs["aev"], np.float32).reshape(-1, DIN))
    order = np.argsort(species, kind="stable")
    counts = np.bincount(species, minlength=4)
    loads = []
    for s in range(4):
        h = (int(counts[s]) + 1) // 2
        loads += [h, int(counts[s]) - h]
    A_pc = max(512, int(np.ceil(max(loads) / QUANTUM)) * QUANTUM)

    idx = np.full((NCORES, A_pc), -1, dtype=np.int64)
    off = 0
    for s in range(4):
        grp = order[off:off + counts[s]]
        off += counts[s]
        h = (int(counts[s]) + 1) // 2
        idx[2 * s, :h] = grp[:h]
        idx[2 * s + 1, :counts[s] - h] = grp[h:]

    import ml_dtypes
    aev_t = np.zeros((NCORES, DIN, A_pc), dtype=ml_dtypes.bfloat16)
    for c in range(NCORES):
        valid = idx[c] >= 0
        aev_t[c][:, valid] = aev[idx[c][valid]].T.astype(ml_dtypes.bfloat16)

    sched = _make_sched(A_pc)
    return aev_t, idx, sched, A_pc


def _build_program(sched, A_pc):
    nc = bacc.Bacc("TRN2", target_bir_lowering=False, debug=False)
    aev_d = nc.dram_tensor("aev_t", [DIN, A_pc], BF16, kind="ExternalInput").ap()
    w_d = nc.dram_tensor("wimg", [P, WCOLS], BF16, kind="ExternalInput").ap()
    out_d = nc.dram_tensor("out", [1, A_pc], F32, kind="ExternalOutput").ap()
    T = len(sched)

    with tile.TileContext(nc) as tc, ExitStack() as ctx:
        spool = ctx.enter_context(tc.tile_pool(name="s", bufs=1))
        xpool = ctx.enter_context(tc.tile_pool(name="x", bufs=5))
        hpool = ctx.enter_context(tc.tile_pool(name="h", bufs=3))
        epool = ctx.enter_context(tc.tile_pool(name="e", bufs=2))
        pspool = ctx.enter_context(tc.tile_pool(name="ps", bufs=1, space="PSUM"))

        wt = spool.tile([P, WCOLS], BF16, tag="wt")
        nc.sync.dma_start(wt[:, :WSPLIT], w_d[:, :WSPLIT])

        ones = spool.tile([P, 512], BF16, tag="ones")
        nc.vector.memset(ones[:], 1.0)
        hm1 = [spool.tile([P, 512], BF16, tag=f"hm1{i}", name=f"hm1{i}")
               for i in range(2)]
        nc.vector.memset(hm1[0][:], 1.0)
        nc.vector.memset(hm1[1][:], 1.0)
        ystage = spool.tile([1, A_pc], F32, tag="ystage")
        # touch Exp so the ACT table load is off the first tile's critical path
        scratch = spool.tile([1, 1], F32, tag="scratch")
        nc.scalar.activation(scratch[:], ones[0:1, 0:1], AF.Exp,
                             bias=0.0, scale=1.0)

        xloads = {}

        def stage_load(u):
            col, n = sched[u]
            xt = xpool.tile([P, 3, 512], BF16, tag="x")
            src = aev_d.rearrange("(k p) a -> p k a", k=3)
            nc.sync.dma_start(xt[:, :, :n], src[:, :, col:col + n])
            xloads[u] = xt

        stage_load(0)
        if T > 1:
            stage_load(1)
        nc.sync.dma_start(wt[:, WSPLIT:], w_d[:, WSPLIT:])

        psm1 = pspool.tile([P, 2, 512], F32, tag="psm1")
        hid = {}      # (tile, layer) -> h tile
        h23_of = {}   # iteration -> h23 tile
        MM = nc.tensor.matmul

        def emit_augs(i, ps1, ps23):
            """One ROW-mode slot: up to 4 concurrent K=1 aug matmuls at rows
            0/32/64/96, each into a different PSUM bank, start=True (clears
            their banks; mains follow with start=False)."""
            t4, t2 = i + 4, i + 2
            if ps1 is not None:
                _, n4 = sched[t4]
                MM(ps1[:, 0, :n4], wt[0:1, AUG:AUG + P], ones[0:1, :n4],
                   start=True, stop=False, tile_position=(0, 0))
                MM(ps1[:, 1, :n4], wt[32:33, AUG:AUG + P], ones[32:33, :n4],
                   start=True, stop=False, tile_position=(32, 0))
            if 0 <= t2 < T:
                _, n2 = sched[t2]
                MM(ps23[:, 0, :n2], wt[64:65, AUG:AUG + P], ones[64:65, :n2],
                   start=True, stop=False, tile_position=(64, 0))

        def emit_l1_mains(i, ps1):
            t4 = i + 4
            _, n4 = sched[t4]
            xt = xloads.pop(t4)
            for m in range(2):
                for k in range(3):
                    MM(ps1[:, m, :n4], wt[:, L1C + (m * 3 + k) * P:
                                          L1C + (m * 3 + k + 1) * P],
                       xt[:, k, :n4], start=False, stop=(k == 2))

        def emit_full_23(i, ps23):
            """FULL-mode L2m0(t2) + L3m0(t1) including hm1-ride k1."""
            t2, t1 = i + 2, i + 1
            if 0 <= t2 < T:
                _, n2 = sched[t2]
                h1 = hid[(t2, 1)]
                for k in range(2):
                    MM(ps23[:, 0, :n2], wt[:, L2C + k * P:L2C + (k + 1) * P],
                       h1[:, k, :n2], start=False, stop=(k == 1))
            if 0 <= t1 < T:
                _, n1 = sched[t1]
                h23p = h23_of[i - 1]
                MM(ps23[:, 1, :n1], wt[:, L3K0:L3K0 + P], h23p[:, 0, :n1],
                   start=True, stop=False)
                MM(ps23[:, 1, :n1], wt[:, FK1:FK1 + P],
                   hm1[(i - 1) % 2][:, :n1], start=False, stop=True)

        def emit_col(i):
            """COL-mode trio: L2m1(t2)@64:128, L3m1(t1)@32:64, L4(t0)@0:1."""
            t2, t1, t0 = i + 2, i + 1, i
            b = i % 2
            if 0 <= t2 < T:
                _, n2 = sched[t2]
                # L2m1 aug: K=1 weight at row 0, pure col-mode, starts the
                # [64:128] accumulation ranges
                MM(psm1[64:96, b, :n2], wt[0:1, A2M1:A2M1 + 32],
                   ones[0:1, :n2], start=True, stop=False,
                   tile_position=(0, 64))
                MM(psm1[96:128, b, :n2], wt[0:1, A2M1 + 32:A2M1 + 64],
                   ones[0:1, :n2], start=True, stop=False,
                   tile_position=(0, 96))
            if 0 <= t1 < T:
                _, n1 = sched[t1]
                h23p = h23_of[i - 1]
                MM(psm1[32:64, b, :n1], wt[:, GK0:GK0 + 32], h23p[:, 0, :n1],
                   start=True, stop=False, tile_position=(0, 32))
            if 0 <= t0 < T:
                _, n0 = sched[t0]
                h23p = h23_of[i - 1]
                MM(psm1[0:32, b, :n0], wt[:, IK0:IK0 + 32], h23p[:, 1, :n0],
                   start=True, stop=False, tile_position=(0, 0))
            else:
                # warmup: zero psm1[0:32] so the full-span stt makes exact
                # 1.0 pad rows (rides live on hm1 row 1)
                nz = max(sched[t][1] for t in (t2, t1) if 0 <= t < T)
                MM(psm1[0:32, b, :nz], wt[:, Z32:Z32 + 32], ones[:, :nz],
                   start=True, stop=True, tile_position=(0, 0))
            if 0 <= t2 < T:
                _, n2 = sched[t2]
                h1 = hid[(t2, 1)]
                MM(psm1[64:96, b, :n2], wt[:, EK0:EK0 + 32], h1[:, 0, :n2],
                   start=False, stop=False, tile_position=(0, 64))
                MM(psm1[96:128, b, :n2], wt[:, EK0 + 32:EK0 + 64],
                   h1[:, 0, :n2], start=False, stop=False,
                   tile_position=(0, 96))
            # k1 members (stream hm1: 1.0-rows carry aug3m1 / shifter bias)
            if 0 <= t1 < T:
                _, n1 = sched[t1]
                MM(psm1[32:64, b, :n1], wt[:, GK1:GK1 + 32],
                   hm1[(i - 1) % 2][:, :n1], start=False, stop=True,
                   tile_position=(0, 32))
            if 0 <= t0 < T:
                _, n0 = sched[t0]
                MM(psm1[0:32, b, :n0], wt[:, IK1:IK1 + 32],
                   hm1[(i - 1) % 2][:, :n0], start=False, stop=True,
                   tile_position=(0, 0))
            if 0 <= t2 < T:
                _, n2 = sched[t2]
                h1 = hid.pop((t2, 1))
                MM(psm1[64:96, b, :n2], wt[:, EK1:EK1 + 32], h1[:, 1, :n2],
                   start=False, stop=True, tile_position=(0, 64))
                MM(psm1[96:128, b, :n2], wt[:, EK1 + 32:EK1 + 64],
                   h1[:, 1, :n2], start=False, stop=True,
                   tile_position=(0, 96))

        def post_m1(i):
            """Exp + stt over the full psm1 bank -> hm1[i%2].  Rows 1:32
            are zero in psum, so stt yields exact 1.0 pad rows there (the
            bias rides); row 0 (L4 out) becomes garbage in hm1 but is only
            ever streamed against zero weight rows."""
            b = i % 2
            em1 = epool.tile([P, 512], BF16, tag="em1")
            nc.scalar.activation(em1[:, :], psm1[:, b, :], AF.Exp,
                                 bias=0.0, scale=1.0 / E1)
            nc.vector.scalar_tensor_tensor(
                hm1[b][:, :], em1[:, :], E1, psm1[:, b, :],
                ALU.min, ALU.max)

        def post_23(i, ps23):
            t2, t1 = i + 2, i + 1
            b0 = 0 if 0 <= t2 < T else 1
            b1 = 2 if 0 <= t1 < T else 1
            if b0 >= b1:
                return
            n = max(sched[t][1] for t in (t2, t1) if 0 <= t < T)
            e23 = epool.tile([P, 2, 512], BF16, tag="e23")
            nc.scalar.activation(e23[:, b0:b1, :n], ps23[:, b0:b1, :n], AF.Exp,
                                 bias=0.0, scale=1.0 / E1)
            h23 = hpool.tile([P, 2, 512], BF16, tag="h23")
            nc.vector.scalar_tensor_tensor(
                h23[:, b0:b1, :n], e23[:, b0:b1, :n], E1, ps23[:, b0:b1, :n],
                ALU.min, ALU.max)
            h23_of[i] = h23

        def post_l1(i, ps1):
            t4 = i + 4
            _, n4 = sched[t4]
            e1 = epool.tile([P, 2, 512], BF16, tag="e1")
            nc.scalar.activation(e1[:, :, :n4], ps1[:, :, :n4], AF.Exp,
                                 bias=0.0, scale=1.0 / E1)
            h1 = hpool.tile([P, 2, 512], BF16, tag="h1")
            nc.vector.scalar_tensor_tensor(
                h1[:, :, :n4], e1[:, :, :n4], E1, ps1[:, :, :n4],
                ALU.min, ALU.max)
            hid[(t4, 1)] = h1

        def evac(i):
            col, n = sched[i]
            src = psm1[0:1, i % 2, :n]
            dst = ystage[0:1, col:col + n]
            if i % 2 == 0:
                nc.vector.tensor_scalar_add(dst, src, 0.0)
            else:
                nc.scalar.activation(dst, src, AF.Identity, bias=0.0,
                                     scale=1.0)
            nc.sync.dma_start(out_d[:, col:col + n], ystage[:, col:col + n])

        for i in range(-4, T):
            t4, t2, t1, t0 = i + 4, i + 2, i + 1, i
            if 2 <= i + 6 < T:
                stage_load(i + 6)
            ps1 = None
            if 0 <= t4 < T:
                ps1 = pspool.tile([P, 2, 512], F32, tag="ps1", bufs=2)
            ps23 = None
            if 0 <= t2 < T or 0 <= t1 < T:
                ps23 = pspool.tile([P, 2, 512], F32, tag="ps23")
            emit_augs(i, ps1, ps23)
            if i < 0:
                # warmup: L1 first so its ready matmuls aren't queued behind
                # an L2 that waits on h1
                if ps1 is not None:
                    emit_l1_mains(i, ps1)
                if ps23 is not None:
                    emit_full_23(i, ps23)
            else:
                if ps23 is not None:
                    emit_full_23(i, ps23)
                if ps1 is not None:
                    emit_l1_mains(i, ps1)
            if 0 <= t2 < T or 0 <= t1 < T or 0 <= t0 < T:
                emit_col(i)
            if ps23 is not None:
                post_23(i, ps23)
            if 0 <= t2 < T or 0 <= t1 < T:
                post_m1(i)
            if ps1 is not None:
                post_l1(i, ps1)
            if 0 <= t0 < T:
                evac(i)

    nc.compile()
    return nc


def kernel(**inputs):
    import ml_dtypes
    species = np.asarray(inputs["species"])
    out_dtype = np.asarray(inputs["aev"]).dtype
    aev_t, idx, sched, A_pc = _host_prepare(inputs)
    wimgs = _fold_host(inputs)
    nc = _build_program(sched, A_pc)

    in_maps = [{"aev_t": np.ascontiguousarray(aev_t[c]),
                "wimg": wimgs[c // 2].astype(ml_dtypes.bfloat16)}
               for c in range(NCORES)]
    res = run_bass_kernel_spmd(nc, in_maps, core_ids=list(range(NCORES)))

    out = np.zeros(species.size, dtype=np.float32)
    for c in range(NCORES):
        valid = idx[c] >= 0
        out[idx[c][valid]] = res.results[c]["out"][0][valid]
    return out.reshape(species.shape).astype(out_dtype, copy=False)


# revision 12
# speedup vs baseline: 1.1124x; 1.0143x over previous
# Trainium2 Bass kernel for nn_CoefficientLayer (per-species MLP dispatch,
# ANI-style).
#
# v5: species s -> cores 2s, 2s+1 (routing and weight-folding as v4).  The
# device schedule is rebuilt around three ideas:
#
# 1. PE slot packing (13 N=512 slots/tile, was 16, grouped by array mode to
#    minimize reconfig drains):
#      ROW slot : 4 K=1 bias ("aug") matmuls at row groups 0/32/64/96
#                 (L1 m0/m1, L2 m0/m1), each into a different PSUM bank,
#                 emitted FIRST with start=True so they clear their banks.
#      FULL x10 : L2m0 k0/k1; L3m0 k0; L3m0 k1 zero-padded to K=128 so it
#                 streams the shared `hm1` tile (rows 0:32 are constant 1.0
#                 -> L3's bias rides weight row 0 for free); L1 x6.
#      COL  x2  : column-tiled trio sharing one PSUM bank at disjoint
#                 partitions: L4 -> [0:1] (col 0), L3m1 -> [32:64] (col 32),
#                 L2m1 -> [64:128] (col 64).  k1 members stream `hm1`
#                 (1.0-rows carry L3m1's bias and L4's folded shifter bias).
# 2. psm1: one persistent [P,2,512] f32 tile (2 banks, even/odd tiles) holds
#    L2m1(t+2) + L3m1(t+1) + L4(t); a single Exp/stt pair post-processes
#    [32:128]; the L4 row [0:1] is evacuated to SBUF by a copy that
#    alternates ScalarE/VectorE per tile to balance engine load.
# 3. Merged posts: L2m0(t+2) and L3m0(t+1) share one [P,2,512] psum tile so
#    one Exp + one stt covers both (saves inst overhead on both engines).
#
# Math (e = exp(1)): stored Hb_k = e*(elu(y_k)+1), y_k = x_k/alpha:
#   psum = e*(y+1) (weight chunks + aug rank-1 updates)
#   ACT:  E = Exp(psum/e);   DVE:  Hb = (E min e) max psum
# L4: psum4 = (s1*alpha/e)*W4^T Hb3 + ride(s0 + s1*beta4) = final output.
import numpy as np
from contextlib import ExitStack

import concourse.bass as bass
import concourse.tile as tile
from concourse import bacc, mybir
from concourse.bass_utils import run_bass_kernel_spmd

ALPHA = 0.1
E1 = float(np.exp(1.0))
P = 128
NCORES = 8
QUANTUM = 128
DIN = 384

F32 = mybir.dt.float32
BF16 = mybir.dt.bfloat16
AF = mybir.ActivationFunctionType
ALU = mybir.AluOpType

# weight image column layout (per species, [128, WCOLS] bf16)
AUG = 0            # 128 cols: row0=aug1[0:128] row32=aug1[128:256]
                   #           row64=aug2[0:128] row96=aug2[128:192] (64 cols)
L1C = 128          # 6 x 128: (m,k) -> L1C + (m*3+k)*128
L2C = L1C + 6 * P  # 2 x 128: L2m0 k
L3K0 = L2C + 2 * P   # 128: W3p[0:128, 0:128]
FK1 = L3K0 + P       # 128: row1=aug3[0:128], rows64:128=W3p[128:192, 0:128]
EK0 = FK1 + P        # 64: W2p[0:128, 128:192]
EK1 = EK0 + 64       # 64: W2p[128:256, 128:192]
GK0 = EK1 + 64       # 32: W3p[0:128, 128:160]
GK1 = GK0 + 32       # 32: row1=aug3[128:160], rows64:128=W3p[128:192,128:160]
IK0 = GK1 + 32       # 32: col0=W4p[0:128], cols1:32 zero (zeros psm1[1:32])
IK1 = IK0 + 32       # 32: col0: row1=s0+s1*beta4, rows32:64=W4p[128:160]
A2M1 = IK1 + 32      # 64: row0=aug2[128:192] (two 32-col halves, col-mode)
Z32 = A2M1 + 64      # 32 zero cols (warmup dummy clear of psm1[0:32])
WCOLS = Z32 + 32
WSPLIT = L2C       # prologue DMA-A covers [0:WSPLIT) = AUG + L1


def _fold_host(inputs):
    """Per-species weight image [4][128, WCOLS] f32."""
    al = ALPHA
    wimgs = []
    for s in range(4):
        W = [np.asarray(inputs[f"W{i}"][s], np.float32) for i in (1, 2, 3, 4)]
        b = [np.asarray(inputs[f"b{i}"][s], np.float32) for i in (1, 2, 3, 4)]
        s1 = float(np.asarray(inputs["shift_b1"], np.float32)[s])
        s0 = float(np.asarray(inputs["shift_b0"], np.float32)[s])
        W1p = (E1 / al) * W[0]
        W2p, W3p = W[1], W[2]
        W4p = (s1 * al / E1) * W[3]
        aug1 = E1 * (b[0] / al + 1.0)
        aug2 = E1 * (b[1] / al - W[1].sum(0) + 1.0)
        aug3 = E1 * (b[2] / al - W[2].sum(0) + 1.0)
        bias4 = s0 + s1 * float((b[3] - al * W[3].sum(0))[0])

        w = np.zeros((P, WCOLS), dtype=np.float32)
        w[0, AUG:AUG + 128] = aug1[:128]
        w[32, AUG:AUG + 128] = aug1[128:]
        w[64, AUG:AUG + 128] = aug2[:128]
        w[0, A2M1:A2M1 + 64] = aug2[128:]
        for m in range(2):
            for k in range(3):
                c = L1C + (m * 3 + k) * P
                w[:, c:c + P] = W1p[k * P:(k + 1) * P, m * P:(m + 1) * P]
        for k in range(2):
            w[:, L2C + k * P:L2C + (k + 1) * P] = W2p[k * P:(k + 1) * P, :P]
        w[:, L3K0:L3K0 + P] = W3p[:P, :P]
        w[1, FK1:FK1 + P] = aug3[:128]
        w[64:, FK1:FK1 + P] = W3p[P:, :P]
        w[:, EK0:EK0 + 64] = W2p[:P, P:]
        w[:, EK1:EK1 + 64] = W2p[P:, P:]
        w[:, GK0:GK0 + 32] = W3p[:P, P:]
        w[1, GK1:GK1 + 32] = aug3[128:]
        w[64:, GK1:GK1 + 32] = W3p[P:, P:]
        w[:, IK0] = W4p[:P, 0]
        w[1, IK1] = bias4
        w[32:64, IK1] = W4p[P:, 0]
        wimgs.append(w)
    return wimgs


def _make_sched(A_pc):
    """Non-increasing tile sizes: 512s then one runt."""
    tiles, col, rem = [], 0, A_pc
    while rem >= 512:
        tiles.append((col, 512))
        col += 512
        rem -= 512
    if rem:
        tiles.append((col, rem))
    return tiles


def _host_prepare(inputs):
    species = np.asarray(inputs["species"]).ravel()
    aev = np.ascontiguousarray(
        np.asarray(inputs["aev"], np.float32).reshape(-1, DIN))
    order = np.argsort(species, kind="stable")
    counts = np.bincount(species, minlength=4)
    loads = []
    for s in range(4):
        h = (int(counts[s]) + 1) // 2
        loads += [h, int(counts[s]) - h]
    A_pc = max(512, int(np.ceil(max(loads) / QUANTUM)) * QUANTUM)

    idx = np.full((NCORES, A_pc), -1, dtype=np.int64)
    off = 0
    for s in range(4):
        grp = order[off:off + counts[s]]
        off += counts[s]
        h = (int(counts[s]) + 1) // 2
        idx[2 * s, :h] = grp[:h]
        idx[2 * s + 1, :counts[s] - h] = grp[h:]

    import ml_dtypes
    aev_t = np.zeros((NCORES, DIN, A_pc), dtype=ml_dtypes.bfloat16)
    for c in range(NCORES):
        valid = idx[c] >= 0
        aev_t[c][:, valid] = aev[idx[c][valid]].T.astype(ml_dtypes.bfloat16)

    sched = _make_sched(A_pc)
    return aev_t, idx, sched, A_pc


def _build_program(sched, A_pc):
    nc = bacc.Bacc("TRN2", target_bir_lowering=False, debug=False)
    aev_d = nc.dram_tensor("aev_t", [DIN, A_pc], BF16, kind="ExternalInput").ap()
    w_d = nc.dram_tensor("wimg", [P, WCOLS], BF16, kind="ExternalInput").ap()
    out_d = nc.dram_tensor("out", [1, A_pc], F32, kind="ExternalOutput").ap()
    T = len(sched)

    with tile.TileContext(nc) as tc, ExitStack() as ctx:
        spool = ctx.enter_context(tc.tile_pool(name="s", bufs=1))
        xpool = ctx.enter_context(tc.tile_pool(name="x", bufs=5))
        hpool = ctx.enter_context(tc.tile_pool(name="h", bufs=3))
        epool = ctx.enter_context(tc.tile_pool(name="e", bufs=2))
        pspool = ctx.enter_context(tc.tile_pool(name="ps", bufs=1, space="PSUM"))

        wt = spool.tile([P, WCOLS], BF16, tag="wt")
        nc.sync.dma_start(wt[:, :L1C + 3 * P], w_d[:, :L1C + 3 * P])

        ones = spool.tile([P, 512], BF16, tag="ones")
        nc.vector.memset(ones[:], 1.0)
        hm1 = [spool.tile([P, 512], BF16, tag=f"hm1{i}", name=f"hm1{i}")
               for i in range(2)]
        nc.vector.memset(hm1[0][:], 1.0)
        nc.vector.memset(hm1[1][:], 1.0)
        ystage = spool.tile([1, A_pc], F32, tag="ystage")
        # touch Exp so the ACT table load is off the first tile's critical path
        scratch = spool.tile([1, 1], F32, tag="scratch")
        nc.scalar.activation(scratch[:], ones[0:1, 0:1], AF.Exp,
                             bias=0.0, scale=1.0)

        xloads = {}

        def stage_load(u):
            col, n = sched[u]
            xt = xpool.tile([P, 3, 512], BF16, tag="x")
            src = aev_d.rearrange("(k p) a -> p k a", k=3)
            nc.sync.dma_start(xt[:, :, :n], src[:, :, col:col + n])
            xloads[u] = xt

        stage_load(0)
        nc.sync.dma_start(wt[:, L1C + 3 * P:WSPLIT], w_d[:, L1C + 3 * P:WSPLIT])
        if T > 1:
            stage_load(1)
        nc.sync.dma_start(wt[:, WSPLIT:], w_d[:, WSPLIT:])

        psm1 = pspool.tile([P, 2, 512], F32, tag="psm1")
        hid = {}      # (tile, layer) -> h tile
        h23_of = {}   # iteration -> h23 tile
        MM = nc.tensor.matmul

        def emit_augs(i, ps1, ps23):
            """One ROW-mode slot: up to 4 concurrent K=1 aug matmuls at rows
            0/32/64/96, each into a different PSUM bank, start=True (clears
            their banks; mains follow with start=False)."""
            t4, t2 = i + 4, i + 2
            if ps1 is not None:
                _, n4 = sched[t4]
                MM(ps1[:, 0, :n4], wt[0:1, AUG:AUG + P], ones[0:1, :n4],
                   start=True, stop=False, tile_position=(0, 0))
                MM(ps1[:, 1, :n4], wt[32:33, AUG:AUG + P], ones[32:33, :n4],
                   start=True, stop=False, tile_position=(32, 0))
            if 0 <= t2 < T:
                _, n2 = sched[t2]
                MM(ps23[:, 0, :n2], wt[64:65, AUG:AUG + P], ones[64:65, :n2],
                   start=True, stop=False, tile_position=(64, 0))

        def emit_l1_mains(i, ps1):
            t4 = i + 4
            _, n4 = sched[t4]
            xt = xloads.pop(t4)
            for m in range(2):
                for k in range(3):
                    MM(ps1[:, m, :n4], wt[:, L1C + (m * 3 + k) * P:
                                          L1C + (m * 3 + k + 1) * P],
                       xt[:, k, :n4], start=False, stop=(k == 2))

        def emit_full_23(i, ps23):
            """FULL-mode L3m0(t1) (incl hm1-ride k1) then L2m0(t2)."""
            t2, t1 = i + 2, i + 1
            if 0 <= t1 < T:
                _, n1 = sched[t1]
                h23p = h23_of[i - 1]
                MM(ps23[:, 1, :n1], wt[:, L3K0:L3K0 + P], h23p[:, 0, :n1],
                   start=True, stop=False)
                MM(ps23[:, 1, :n1], wt[:, FK1:FK1 + P],
                   hm1[(i - 1) % 2][:, :n1], start=False, stop=True)
            if 0 <= t2 < T:
                _, n2 = sched[t2]
                h1 = hid.pop((t2, 1))
                for k in range(2):
                    MM(ps23[:, 0, :n2], wt[:, L2C + k * P:L2C + (k + 1) * P],
                       h1[:, k, :n2], start=False, stop=(k == 1))

        def emit_col(i):
            """COL-mode phase: a2m1 aug pair, then k1 quad (streams hm1,
            ready early), then k0 quad (streams h23, ready later).  k1
            members carry start=True for their ranges; k0 members stop."""
            t2, t1, t0 = i + 2, i + 1, i
            b = i % 2
            if 0 <= t2 < T:
                _, n2 = sched[t2]
                MM(psm1[64:96, b, :n2], wt[0:1, A2M1:A2M1 + 32],
                   ones[0:1, :n2], start=True, stop=False,
                   tile_position=(0, 64))
                MM(psm1[96:128, b, :n2], wt[0:1, A2M1 + 32:A2M1 + 64],
                   ones[0:1, :n2], start=True, stop=False,
                   tile_position=(0, 96))
            # k1 quad (stream hm1: 1.0-rows carry aug3m1 / shifter bias)
            if 0 <= t1 < T:
                _, n1 = sched[t1]
                MM(psm1[32:64, b, :n1], wt[:, GK1:GK1 + 32],
                   hm1[(i - 1) % 2][:, :n1], start=True, stop=False,
                   tile_position=(0, 32))
            if 0 <= t0 < T:
                _, n0 = sched[t0]
                MM(psm1[0:32, b, :n0], wt[:, IK1:IK1 + 32],
                   hm1[(i - 1) % 2][:, :n0], start=True, stop=False,
                   tile_position=(0, 0))
            else:
                # warmup: zero psm1[0:32] so the full-span stt makes exact
                # 1.0 pad rows (rides live on hm1 row 1)
                nz = max(sched[t][1] for t in (t2, t1) if 0 <= t < T)
                MM(psm1[0:32, b, :nz], wt[:, Z32:Z32 + 32], ones[:, :nz],
                   start=True, stop=True, tile_position=(0, 0))
            if 0 <= t2 < T:
                _, n2 = sched[t2]
                h1 = hid[(t2, 1)]
                MM(psm1[64:96, b, :n2], wt[:, EK1:EK1 + 32], h1[:, 1, :n2],
                   start=False, stop=False, tile_position=(0, 64))
                MM(psm1[96:128, b, :n2], wt[:, EK1 + 32:EK1 + 64],
                   h1[:, 1, :n2], start=False, stop=False,
                   tile_position=(0, 96))
            # k0 quad (streams h23 from the previous iteration's stt23)
            if 0 <= t1 < T:
                _, n1 = sched[t1]
                h23p = h23_of[i - 1]
                MM(psm1[32:64, b, :n1], wt[:, GK0:GK0 + 32], h23p[:, 0, :n1],
                   start=False, stop=True, tile_position=(0, 32))
            if 0 <= t0 < T:
                _, n0 = sched[t0]
                h23p = h23_of[i - 1]
                MM(psm1[0:32, b, :n0], wt[:, IK0:IK0 + 32], h23p[:, 1, :n0],
                   start=False, stop=True, tile_position=(0, 0))
            if 0 <= t2 < T:
                _, n2 = sched[t2]
                h1 = hid[(t2, 1)]
                MM(psm1[64:96, b, :n2], wt[:, EK0:EK0 + 32], h1[:, 0, :n2],
                   start=False, stop=True, tile_position=(0, 64))
                MM(psm1[96:128, b, :n2], wt[:, EK0 + 32:EK0 + 64],
                   h1[:, 0, :n2], start=False, stop=True,
                   tile_position=(0, 96))

        def post_m1(i):
            """Exp + stt over the full psm1 bank -> hm1[i%2].  Rows 1:32
            are zero in psum, so stt yields exact 1.0 pad rows there (the
            bias rides); row 0 (L4 out) becomes garbage in hm1 but is only
            ever streamed against zero weight rows."""
            b = i % 2
            em1 = epool.tile([P, 512], BF16, tag="em1")
            nc.scalar.activation(em1[:, :], psm1[:, b, :], AF.Exp,
                                 bias=0.0, scale=1.0 / E1)
            nc.vector.scalar_tensor_tensor(
                hm1[b][:, :], em1[:, :], E1, psm1[:, b, :],
                ALU.min, ALU.max)

        def post_23(i, ps23):
            t2, t1 = i + 2, i + 1
            b0 = 0 if 0 <= t2 < T else 1
            b1 = 2 if 0 <= t1 < T else 1
            if b0 >= b1:
                return
            n = max(sched[t][1] for t in (t2, t1) if 0 <= t < T)
            e23 = epool.tile([P, 2, 512], BF16, tag="e23")
            nc.scalar.activation(e23[:, b0:b1, :n], ps23[:, b0:b1, :n], AF.Exp,
                                 bias=0.0, scale=1.0 / E1)
            h23 = hpool.tile([P, 2, 512], BF16, tag="h23")
            nc.vector.scalar_tensor_tensor(
                h23[:, b0:b1, :n], e23[:, b0:b1, :n], E1, ps23[:, b0:b1, :n],
                ALU.min, ALU.max)
            h23_of[i] = h23

        def post_l1(i, ps1):
            t4 = i + 4
            _, n4 = sched[t4]
            e1 = epool.tile([P, 2, 512], BF16, tag="e1")
            nc.scalar.activation(e1[:, :, :n4], ps1[:, :, :n4], AF.Exp,
                                 bias=0.0, scale=1.0 / E1)
            h1 = hpool.tile([P, 2, 512], BF16, tag="h1")
            nc.vector.scalar_tensor_tensor(
                h1[:, :, :n4], e1[:, :, :n4], E1, ps1[:, :, :n4],
                ALU.min, ALU.max)
            hid[(t4, 1)] = h1

        def evac(i):
            col, n = sched[i]
            src = psm1[0:1, i % 2, :n]
            dst = ystage[0:1, col:col + n]
            if i % 2 == 0:
                nc.vector.tensor_scalar_add(dst, src, 0.0)
            else:
                nc.scalar.activation(dst, src, AF.Identity, bias=0.0,
                                     scale=1.0)
            nc.sync.dma_start(out_d[:, col:col + n], ystage[:, col:col + n])

        for i in range(-4, T):
            t4, t2, t1, t0 = i + 4, i + 2, i + 1, i
            if 2 <= i + 6 < T:
                stage_load(i + 6)
            ps1 = None
            if 0 <= t4 < T:
                ps1 = pspool.tile([P, 2, 512], F32, tag="ps1", bufs=2)
            ps23 = None
            if 0 <= t2 < T or 0 <= t1 < T:
                ps23 = pspool.tile([P, 2, 512], F32, tag="ps23")
            emit_augs(i, ps1, ps23)
            if i < 0:
                # warmup: L1 first so its ready matmuls aren't queued behind
                # stages that wait on not-yet-computed h tiles
                if ps1 is not None:
                    emit_l1_mains(i, ps1)
                if 0 <= t2 < T or 0 <= t1 < T or 0 <= t0 < T:
                    emit_col(i)
                if ps23 is not None:
                    emit_full_23(i, ps23)
            else:
                if 0 <= t2 < T or 0 <= t1 < T or 0 <= t0 < T:
                    emit_col(i)
                if ps23 is not None:
                    emit_full_23(i, ps23)
                if ps1 is not None:
                    emit_l1_mains(i, ps1)
            if 0 <= t2 < T or 0 <= t1 < T:
                post_m1(i)
            if ps23 is not None:
                post_23(i, ps23)
            if ps1 is not None:
                post_l1(i, ps1)
            if 0 <= t0 < T:
                evac(i)

    nc.compile()
    return nc


def kernel(**inputs):
    import ml_dtypes
    species = np.asarray(inputs["species"])
    out_dtype = np.asarray(inputs["aev"]).dtype
    aev_t, idx, sched, A_pc = _host_prepare(inputs)
    wimgs = _fold_host(inputs)
    nc = _build_program(sched, A_pc)

    in_maps = [{"aev_t": np.ascontiguousarray(aev_t[c]),
                "wimg": wimgs[c // 2].astype(ml_dtypes.bfloat16)}
               for c in range(NCORES)]
    res = run_bass_kernel_spmd(nc, in_maps, core_ids=list(range(NCORES)))

    out = np.zeros(species.size, dtype=np.float32)
    for c in range(NCORES):
        valid = idx[c] >= 0
        out[idx[c][valid]] = res.results[c]["out"][0][valid]
    return out.reshape(species.shape).astype(out_dtype, copy=False)
